# revision 61
# baseline (speedup 1.0000x reference)
"""Causal multi-head attention block (B=8, T=2048, C=768, H=8) on 8 trn2 cores.

Sharding: data-parallel over batch — one batch element per NeuronCore, weights
replicated, no collectives.

Per-core design (bf16 matmul inputs, f32 PSUM accumulation):
  - Inputs stream in via casting gpsimd DMAs (f32 -> bf16) and are
    PE-transposed into resident x^T / w_attn^T / w_proj^T SBUF tiles.
  - Everything lives in ONE tile-pool scope (pool closes insert all-engine
    barriers), and the emission order IS the per-engine execution order:
    the V projection interleaves with the x-chunk transposes, each head's
    Q^T/K^T projection is emitted in t-halves so the second half fills the
    previous attention half's exp-wait windows, and weight transposes ride
    inside earlier heads' attention.
  - Attention per head in S^T layout: S^T[j, i] by d-contraction (bf16),
    P = exp(S^T) on ACT (short staircase tiles pair into one exp call),
    diagonal-block mask multiply on gpsimd, O^T (+ ones-row denominator l)
    accumulated in PSUM over j-tiles via lhsT=[V|1]; normalized by 1/l
    (DVE reciprocal of the l row + gpsimd partition_broadcast), spilled to
    DRAM as bf16 per i-half.
  - Output projection reads O^T back as six 128-row contraction stripes
    (full-partition matmuls); tiles tg0/tg1 run inside head 7's ACT-bound
    second half, the rest behind it; out DMAs stream per t-tile on HWDGE.
"""

import math
import os
import sys
from contextlib import ExitStack

for _p in ("/opt/trn_rl_repo", "/root/.axon_site/_ro/trn_rl_repo"):
    if os.path.isdir(_p) and _p not in sys.path:
        sys.path.append(_p)

import numpy as np

import concourse.bass as bass  # noqa: F401  (import keeps bass registered)
from concourse import bacc
import concourse.mybir as mybir
import concourse.tile as tile
from concourse.bass_utils import run_bass_kernel_spmd

F32 = mybir.dt.float32
F32R = mybir.dt.float32r
BF16 = mybir.dt.bfloat16
EXP = mybir.ActivationFunctionType.Exp
ADD = mybir.AluOpType.add
MULT = mybir.AluOpType.mult

B, T, C, H, HS = 8, 2048, 768, 8, 96
KT = C // 128        # 6 contraction tiles of 128
TT = T // 128        # 16 t-tiles of 128
NCORES = 8


def _chunks(lo, hi, align=512):
    """Split [lo, hi) at multiples of `align`."""
    out = []
    a = lo
    while a < hi:
        b = min(hi, (a // align + 1) * align)
        out.append((a, b))
        a = b
    return out


def build_nc():
    nc = bacc.Bacc()
    x_b = nc.dram_tensor("x_b", [T, C], F32, kind="ExternalInput")
    wat = nc.dram_tensor("wat", [3 * C, C], F32, kind="ExternalInput")
    wp = nc.dram_tensor("wp", [C, C], F32, kind="ExternalInput")
    ident = nc.dram_tensor("ident", [128, 128], BF16, kind="ExternalInput")
    mk = nc.dram_tensor("mk", [128, 128], F32, kind="ExternalInput")
    bqk = nc.dram_tensor("bqk", [HS, 16], F32, kind="ExternalInput")
    bv = nc.dram_tensor("bv", [128, C], F32, kind="ExternalInput")
    bo = nc.dram_tensor("bo", [128, C], F32, kind="ExternalInput")
    out = nc.dram_tensor("out", [T, C], F32, kind="ExternalOutput")

    with tile.TileContext(nc) as tc, ExitStack() as ctx:
        # Single flat pool scope: tile-pool context exits insert all-engine
        # barriers, which serialized the load/transpose phase against the
        # attention phase. Everything lives in one scope so the scheduler can
        # overlap phases freely; PSUM pools are consolidated to fit 8 banks.
        dram = ctx.enter_context(tc.tile_pool(name="dram", bufs=1, space="DRAM"))
        oT = dram.tile([H, HS, T], BF16, tag="oT")

        consts = ctx.enter_context(tc.tile_pool(name="consts", bufs=1))
        id_sb = consts.tile([128, 128], BF16, tag="id")
        mk_sb = consts.tile([128, 128], F32, tag="mk")
        bqk_sb = consts.tile([HS, 16], F32, tag="bqk")
        bv_sb = consts.tile([128, C], F32, tag="bv")
        bo_sb = consts.tile([128, C], F32, tag="bo")
        # ident first: the very first transposes need it (bf16 so transposes
        # run at 1.0 cycles/row; supplied pre-cast by the host)
        nc.sync.dma_start(id_sb[:], ident[:, :])

        xTp = ctx.enter_context(tc.tile_pool(name="xT", bufs=1))
        xT = xTp.tile([128, KT, T], BF16, tag="xT")
        waTp = ctx.enter_context(tc.tile_pool(name="waT", bufs=1))
        waT = waTp.tile([128, KT, 3 * C], BF16, tag="waT")   # w_attn^T resident
        wpTp = ctx.enter_context(tc.tile_pool(name="wpT", bufs=1))
        wpT_sb = wpTp.tile([128, KT, C], BF16, tag="wpTsb")  # w_proj^T resident

        pin = ctx.enter_context(tc.tile_pool(name="pin", bufs=4))
        vsbp = ctx.enter_context(tc.tile_pool(name="vsb", bufs=2))
        qkp = ctx.enter_context(tc.tile_pool(name="qk", bufs=4))
        ptp = ctx.enter_context(tc.tile_pool(name="pt", bufs=3))
        epp = ctx.enter_context(tc.tile_pool(name="ep", bufs=2))
        pcp = ctx.enter_context(tc.tile_pool(name="pc", bufs=2))
        pco = ctx.enter_context(tc.tile_pool(name="pco", bufs=4))
        # PSUM: pjps 2x[128,512] (transposes + projections), bps 2x[128,1024]
        # (S tiles, epilogue broadcast, phase-C accumulation), opsp 1x[128,1024]
        # (O^T accumulator) = 8 banks exactly.
        pjps = ctx.enter_context(tc.tile_pool(name="pj", bufs=2, space="PSUM"))
        bps = ctx.enter_context(tc.tile_pool(name="bps", bufs=2, space="PSUM"))
        opsp = ctx.enter_context(tc.tile_pool(name="ops", bufs=1, space="PSUM"))

        # ---------------- Loads + transposes ----------------
        # Engines execute their instruction streams IN ORDER, so emission
        # order below is chosen to keep PE saturated: x transposes and the
        # V projection interleave with the streaming loads, head-h+1's QK
        # projection is emitted inside head h's attention, and the w_proj
        # transposes ride along in later heads.
        wat_r = wat.rearrange("(a p) c -> p a c", p=128)
        x_r = x_b.rearrange("(a p) c -> p a c", p=128)
        wp_r = wp.rearrange("(a p) c -> p a c", p=128)

        def tr_group(src, n, kc, dst):
            """Transpose n consecutive 128-blocks of `src` for contraction
            stripe kc into one PSUM tile, then evict (casting to bf16)."""
            psb = bps.tile([128, 1024], BF16, tag="ps", name="tps")
            for i in range(n):
                nc.tensor.matmul(psb[:, i * 128:(i + 1) * 128],
                                 src[:, i, kc * 128:(kc + 1) * 128],
                                 id_sb[:], is_transpose=True,
                                 start=(i == 0), stop=(i == n - 1))
            nc.any.tensor_copy(dst, psb[:, 0:n * 128])

        def tr6(src, n, dst, off):
            for kc in range(KT):
                tr_group(src, n, kc, dst[:, kc, off:off + n * 128])

        def load3(name, src_r, a0, n=3):
            """Load n 128-row blocks [a0, a0+n) of a DRAM tensor."""
            t = pin.tile([128, 3, C], BF16, tag="pin", name=name)
            nc.gpsimd.dma_start(t[:, 0:n, :], src_r[:, a0:a0 + n, :])
            return t

        # DMA queue order = consumption order: w_v first (the V projection
        # starts while x still streams), small consts, then x, then q/k
        # weights, w_proj last.
        x0a = load3("x0a", x_r, 0, 1)
        vch0 = load3("vch0", wat_r, 12)       # w_v for heads 0-3
        nc.sync.dma_start(bv_sb[:], bv[:, :])
        x0b = load3("x0b", x_r, 1, 2)
        xg = [None]
        for g in range(1, 6):
            xg.append(load3(f"x{g}", x_r, 3 * g, 3 if g < 5 else 1))
        nc.sync.dma_start(bqk_sb[:], bqk[:, :])
        nc.sync.dma_start(mk_sb[:], mk[:, :])
        qw0 = load3("qw0", wat_r, 0)
        kw0 = load3("kw0", wat_r, 6)
        vch1 = load3("vch1", wat_r, 15)       # w_v for heads 4-7
        qw1 = load3("qw1", wat_r, 3)
        kw1 = load3("kw1", wat_r, 9)
        nc.sync.dma_start(bo_sb[:], bo[:, :])
        wp0 = load3("wp0", wp_r, 0)
        wp1 = load3("wp1", wp_r, 3)

        # ---------------- Projection / attention helpers ----------------
        Vt = [None, None]

        def v_tile(sb):
            V = vsbp.tile([128, TT, 4, HS + 1], BF16, tag="V")
            nc.vector.memset(V[:, :, :, HS:HS + 1], 1.0)
            Vt[sb] = V

        def v_proj(sb, tts):
            # V = x @ w_v^T + b_v in natural [t, d] layout, 384-wide
            # head-aligned chunks, with an appended ones column per head
            # (softmax denominator).
            V = Vt[sb]
            for tt in tts:
                vps = pjps.tile([128, 512], F32, tag="pj")
                for kc in range(KT):
                    nc.tensor.matmul(vps[:, 0:384],
                                     xT[:, kc, tt * 128:(tt + 1) * 128],
                                     waT[:, kc, 2 * C + 384 * sb:
                                         2 * C + 384 * (sb + 1)],
                                     start=(kc == 0), stop=(kc == KT - 1))
                nc.vector.tensor_tensor(
                    V[:, tt, :, 0:HS],
                    vps[:, 0:384].rearrange("p (h d) -> p h d", d=HS),
                    bv_sb[:, 384 * sb:384 * (sb + 1)]
                        .rearrange("p (h d) -> p h d", d=HS),
                    ADD)

        qk_of = {}

        def qk_proj(h, half=None):
            # Q^T/K^T projection for head h ([d, t] layout, Q pre-scaled by
            # 1/sqrt(hs) via prescaled weights, bias via broadcast add).
            # Emitted in t-halves: attention's first i-half only needs
            # columns 0:1024, so the second half fills its exp-wait windows.
            if half in (None, 0):
                qk_of[h] = [qkp.tile([128, T], BF16, tag="qk", name=f"qk{i}")
                            for i in range(2)]
            qkh = qk_of[h]
            tc4s = range(4) if half is None else (
                range(0, 2) if half == 0 else range(2, 4))
            for tc4 in tc4s:
                for mc in range(2):          # 0 = q, 1 = k
                    wc = h * HS + (0 if mc == 0 else C)
                    pj = pjps.tile([128, 512], F32, tag="pj")
                    for kc in range(KT):
                        nc.tensor.matmul(
                            pj[0:HS, 0:512],
                            waT[:, kc, wc:wc + HS],
                            xT[:, kc, tc4 * 512:(tc4 + 1) * 512],
                            start=(kc == 0), stop=(kc == KT - 1))
                    m_col = h + (0 if mc == 0 else 8)
                    nc.vector.tensor_tensor(
                        qkh[mc][0:HS, tc4 * 512:(tc4 + 1) * 512],
                        pj[0:HS, 0:512],
                        bqk_sb[:, m_col:m_col + 1].to_broadcast([HS, 512]),
                        ADD)

        Oe_of = {}

        def attn_half(h, ihalf, fillers=None):
            # Causal attention in S^T layout: S^T[j, i] via d-contraction;
            # P = exp(S^T) on ACT; diagonal-block mask multiply; O^T (+
            # denominator row l) accumulated in PSUM over j-tiles via
            # lhsT=[V|1]; normalize by 1/l (selector-matmul broadcast).
            qT, kT = qk_of[h]
            V = Vt[h // 4]
            hh = h % 4
            if ihalf == 0:
                Oe_of[h] = epp.tile([HS, T], BF16, tag="Oe", bufs=1,
                                    name=f"Oe{h}")
            Oe = Oe_of[h]
            ibase = 1024 * ihalf
            iend = ibase + 1024
            njt = 8 * (ihalf + 1)
            O_ps = opsp.tile([128, 1024], F32, tag="O")
            # short staircase j-tiles pair up in one S tile / one exp call
            # (ACT dispatch overhead is ~0.3us per activation)
            groups = []
            jt = 0
            while jt < njt:
                ilen = iend - max(128 * jt, ibase)
                if 256 <= ilen <= 512 and 128 * jt >= ibase and jt + 1 < njt:
                    groups.append((jt, jt + 1))
                    jt += 2
                else:
                    groups.append((jt,))
                    jt += 1
            def emit_s(grp):
                # S matmuls + exp + diagonal masks for one j-tile group;
                # returns what the deferred PV stage needs
                S = bps.tile([128, 1024], F32, tag="ps")
                P = ptp.tile([128, 1024], BF16, tag="P")
                parts = []
                off = 0
                for jt in grp:
                    j0 = 128 * jt
                    i0 = max(j0, ibase)
                    ilen = iend - i0
                    for (ra, rb) in _chunks(off, off + ilen):
                        nc.tensor.matmul(
                            S[:, ra:rb],
                            kT[0:HS, j0:j0 + 128],
                            qT[0:HS, i0 + ra - off:i0 + rb - off],
                            start=True, stop=True)
                    parts.append((jt, j0, i0, ilen, off))
                    off += ilen
                nc.scalar.activation(P[:, 0:off], S[:, 0:off], EXP)
                for (jt, j0, i0, ilen, off) in parts:
                    if j0 >= ibase:
                        nc.gpsimd.tensor_tensor(P[:, off:off + 128],
                                                P[:, off:off + 128],
                                                mk_sb[:], MULT)
                return P, parts

            def emit_pv(P, parts):
                for (jt, j0, i0, ilen, off) in parts:
                    for (a, b) in _chunks(i0, iend):
                        ci = a // 512
                        last_jt = min(4 * ci + 3, njt - 1)
                        nc.tensor.matmul(
                            O_ps[0:HS + 1, a - ibase:b - ibase],
                            V[:, jt, hh, :],
                            P[:, off + a - i0:off + b - i0],
                            start=(jt == 0), stop=(jt == last_jt))
                    if fillers and jt in fillers:
                        fillers[jt]()

            def epilogue(ra, rb):
                # normalize [ra, rb) of O_ps by the denominator row
                # (reciprocal of the ones-row sum, broadcast across
                # partitions on gpsimd), then spill that slice
                n = rb - ra
                lt = epp.tile([HS + 1, 1024], F32R, tag="lt", bufs=2)
                nc.vector.tensor_copy(lt[:, 0:n], O_ps[0:HS + 1, ra:rb])
                rr = epp.tile([1, 1024], F32, tag="rr", bufs=2)
                nc.vector.reciprocal(rr[:, 0:n], lt[HS:HS + 1, 0:n])
                R = epp.tile([HS, 1024], F32, tag="R", bufs=2)
                nc.gpsimd.partition_broadcast(R[:, 0:n], rr[:, 0:n])
                nc.gpsimd.tensor_tensor(Oe[:, ibase + ra:ibase + rb],
                                        lt[0:HS, 0:n], R[:, 0:n], MULT)
                # spill eagerly: phase C's stripes unblock while the rest
                # of this head is still computing
                nc.sync.dma_start(oT[h, :, ibase + ra:ibase + rb],
                                  Oe[:, ibase + ra:ibase + rb])

            for grp in groups:
                emit_pv(*emit_s(grp))
            epilogue(0, 1024)

        # ---------------- Output projection (per 2-t-tile block) ----------------
        # O^T streamed back from DRAM as six 128-row contraction stripes
        # (full-partition matmuls: 6 accumulation steps per output chunk
        # instead of 8 96-row ones). Runs on the pj PSUM pool, which is idle
        # once the last projections are done — so tg 0..3 (output rows
        # 0..1024, which only need head 7's first i-half) interleave INTO
        # head 7's ACT-bound second half.
        oT_s = oT.rearrange("h p n -> (h p) n").rearrange("(s q) n -> q s n",
                                                          q=128)
        out_r = out.rearrange("(g a p) c -> p g a c", a=2, p=128)

        def pc_tg(tg):
            otg = pco.tile([128, KT, 256], BF16, tag="otg")
            nc.sync.dma_start(otg[:, 0:5, :],
                              oT_s[:, 0:5, tg * 256:(tg + 1) * 256])
            nc.sync.dma_start(otg[:, 5:6, :],
                              oT_s[:, 5:6, tg * 256:(tg + 1) * 256])
            o_sb = pcp.tile([128, 2, C], F32, tag="osb")
            for ta in range(2):
                for (a, b) in ((0, 512), (512, C)):
                    cps = pjps.tile([128, 512], F32, tag="pj")
                    for s in range(KT):
                        nc.tensor.matmul(cps[:, 0:b - a],
                                         otg[:, s, ta * 128:(ta + 1) * 128],
                                         wpT_sb[:, s, a:b],
                                         start=(s == 0), stop=(s == KT - 1))
                    nc.vector.tensor_tensor(o_sb[:, ta, a:b],
                                            cps[:, 0:b - a],
                                            bo_sb[:, a:b], ADD)
                    if tg == TT // 2 - 1 and ta == 1:
                        # stream the very last tile per chunk: shortens the
                        # final bias-add -> DMA wind-down chain
                        nc.sync.dma_start(out_r[:, tg, ta, a:b],
                                          o_sb[:, ta, a:b])
                if not (tg == TT // 2 - 1 and ta == 1):
                    nc.sync.dma_start(out_r[:, tg, ta], o_sb[:, ta])

        # ---------------- Emission schedule ----------------
        # Lead-in: x transposes interleave with w_v transposes and the V
        # projection of chunks already transposed, so PE tracks the DMA
        # stream with minimal idling.
        v_tile(0)
        # first x block: batch 3 contraction stripes per PSUM tile (single
        # 128-col transposes otherwise ping-pong the evict round-trip)
        for k0 in (0, 3):
            psb = bps.tile([128, 1024], BF16, tag="ps", name="tp0")
            for i in range(3):
                nc.tensor.matmul(psb[:, i * 128:(i + 1) * 128],
                                 x0a[:, 0, (k0 + i) * 128:(k0 + i + 1) * 128],
                                 id_sb[:], is_transpose=True,
                                 start=(i == 0), stop=(i == 2))
            nc.any.tensor_copy(xT[:, k0:k0 + 3, 0:128],
                               psb[:, 0:384].rearrange("p (k c) -> p k c",
                                                       c=128))
        tr6(vch0, 3, waT, 12 * 128)
        v_proj(0, range(0, 1))
        for k0 in (0, 3):
            psb = bps.tile([128, 1024], BF16, tag="ps", name="tp0b")
            for i in range(3):
                for a in range(2):
                    nc.tensor.matmul(
                        psb[:, (2 * i + a) * 128:(2 * i + a + 1) * 128],
                        x0b[:, a, (k0 + i) * 128:(k0 + i + 1) * 128],
                        id_sb[:], is_transpose=True,
                        start=(i == 0 and a == 0), stop=(i == 2 and a == 1))
            nc.any.tensor_copy(xT[:, k0:k0 + 3, 128:384],
                               psb[:, 0:768].rearrange("p (k c) -> p k c",
                                                       c=256))
        v_proj(0, range(1, 3))
        tr6(xg[1], 3, xT, 384)
        v_proj(0, range(3, 6))
        tr6(xg[2], 3, xT, 768)
        v_proj(0, range(6, 9))
        tr6(xg[3], 3, xT, 1152)
        v_proj(0, range(9, 12))
        tr6(xg[4], 3, xT, 1536)
        v_proj(0, range(12, 15))
        tr6(xg[5], 1, xT, 1920)
        v_proj(0, range(15, TT))
        tr6(qw0, 3, waT, 0)
        tr6(kw0, 3, waT, 6 * 128)
        qk_proj(0, 0)

        for h in range(H):
            attn_half(h, 0)
            qk_proj(h, 1)
            if h == 1:
                tr6(vch1, 3, waT, 15 * 128)
                tr6(qw1, 3, waT, 3 * 128)
                tr6(kw1, 3, waT, 9 * 128)
            if h == 2:
                v_tile(1)
                v_proj(1, range(0, 5))
            if h == 3:
                v_proj(1, range(5, 12))
            if h == 4:
                tr6(wp0, 3, wpT_sb, 0)
            if h == 5:
                tr6(wp1, 3, wpT_sb, 384)
            if h == H - 1:
                attn_half(h, 1, fillers={14: lambda: pc_tg(0),
                                         15: lambda: pc_tg(1)})
            else:
                attn_half(h, 1)
            if h == 3:
                v_proj(1, range(12, TT))
            if h < H - 1:
                qk_proj(h + 1, 0)

        # ---------------- Output projection ----------------
        # O^T streamed back from DRAM per 2-t-tile block (pipelined) as six
        # 128-row contraction stripes (full-partition matmuls: 6 accumulation
        # steps per output chunk instead of 8 96-row ones). Stripes 0..4 only
        # depend on heads 0..6, so they prefetch during head 7's attention.
        for tg in range(2, TT // 2):
            pc_tg(tg)

    nc.finalize()
    return nc


_NC_CACHE = {}


def _get_nc():
    if "nc" not in _NC_CACHE:
        _NC_CACHE["nc"] = build_nc()
    return _NC_CACHE["nc"]


def _make_consts(b_attn, b_proj):
    s = 1.0 / math.sqrt(HS)
    bqk = np.empty((HS, 16), dtype=np.float32)
    for m in range(8):
        bqk[:, m] = b_attn[m * HS:(m + 1) * HS] * s
    for m in range(8):
        bqk[:, 8 + m] = b_attn[C + m * HS:C + (m + 1) * HS]
    bv = np.ascontiguousarray(
        np.broadcast_to(b_attn[2 * C:3 * C], (128, C)).astype(np.float32))
    bo = np.ascontiguousarray(
        np.broadcast_to(b_proj, (128, C)).astype(np.float32))
    import ml_dtypes
    ident = np.eye(128, dtype=ml_dtypes.bfloat16)
    mk = np.triu(np.ones((128, 128), dtype=np.float32))
    return bqk, bv, bo, ident, mk


def kernel(x, w_attn, b_attn, w_proj, b_proj, _want_results=False, **run_kwargs):
    x = np.asarray(x, dtype=np.float32)
    w_attn = np.asarray(w_attn, dtype=np.float32)
    b_attn = np.asarray(b_attn, dtype=np.float32)
    w_proj = np.asarray(w_proj, dtype=np.float32)
    b_proj = np.asarray(b_proj, dtype=np.float32)

    s = 1.0 / math.sqrt(HS)
    wat = w_attn.copy()
    wat[0:C, :] *= s            # fold the 1/sqrt(hs) logit scale into Q
    bqk, bv, bo, ident, mk = _make_consts(b_attn, b_proj)

    nc = _get_nc()
    common = dict(wat=wat, wp=w_proj, ident=ident, mk=mk,
                  bqk=bqk, bv=bv, bo=bo)
    in_maps = [dict(x_b=np.ascontiguousarray(x[c]), **common)
               for c in range(NCORES)]
    res = run_bass_kernel_spmd(nc, in_maps, core_ids=list(range(NCORES)),
                               **run_kwargs)
    out = np.stack([res.results[c]["out"] for c in range(NCORES)], axis=0)
    if _want_results:
        return out, res
    return out


if __name__ == "__main__":
    rng = np.random.default_rng(0)
    x = rng.standard_normal((B, T, C), dtype=np.float32)
    w_attn = rng.standard_normal((3 * C, C), dtype=np.float32) / math.sqrt(C)
    b_attn = rng.standard_normal(3 * C).astype(np.float32) * 0.02
    w_proj = rng.standard_normal((C, C), dtype=np.float32) / math.sqrt(C)
    b_proj = rng.standard_normal(C).astype(np.float32) * 0.02
    o = kernel(x, w_attn, b_attn, w_proj, b_proj)
    print("out", o.shape, o.dtype, float(np.abs(o).mean()))



# revision 77
# speedup vs baseline: 1.0008x; 1.0008x over previous
"""Causal multi-head attention block (B=8, T=2048, C=768, H=8) on 8 trn2 cores.

Sharding: data-parallel over batch — one batch element per NeuronCore, weights
replicated, no collectives.

Per-core design (bf16 matmul inputs, f32 PSUM accumulation):
  - Inputs stream in via casting gpsimd DMAs (f32 -> bf16) and are
    PE-transposed into resident x^T / w_attn^T / w_proj^T SBUF tiles.
  - Everything lives in ONE tile-pool scope (pool closes insert all-engine
    barriers), and the emission order IS the per-engine execution order:
    the V projection interleaves with the x-chunk transposes, each head's
    Q^T/K^T projection is emitted in t-halves so the second half fills the
    previous attention half's exp-wait windows, and weight transposes ride
    inside earlier heads' attention.
  - Attention per head in S^T layout: S^T[j, i] by d-contraction (bf16),
    P = exp(S^T) on ACT (short staircase tiles pair into one exp call),
    diagonal-block mask multiply on gpsimd, O^T (+ ones-row denominator l)
    accumulated in PSUM over j-tiles via lhsT=[V|1]; normalized by 1/l
    (DVE reciprocal of the l row + gpsimd partition_broadcast), spilled to
    DRAM as bf16 per i-half.
  - Output projection reads O^T back as six 128-row contraction stripes
    (full-partition matmuls); tiles tg0/tg1 run inside head 7's ACT-bound
    second half, the rest behind it; out DMAs stream per t-tile on HWDGE.
"""

import math
import os
import sys
from contextlib import ExitStack

for _p in ("/opt/trn_rl_repo", "/root/.axon_site/_ro/trn_rl_repo"):
    if os.path.isdir(_p) and _p not in sys.path:
        sys.path.append(_p)

import numpy as np

import concourse.bass as bass  # noqa: F401  (import keeps bass registered)
from concourse import bacc
import concourse.mybir as mybir
import concourse.tile as tile
from concourse.bass_utils import run_bass_kernel_spmd

F32 = mybir.dt.float32
F32R = mybir.dt.float32r
BF16 = mybir.dt.bfloat16
EXP = mybir.ActivationFunctionType.Exp
ADD = mybir.AluOpType.add
MULT = mybir.AluOpType.mult

B, T, C, H, HS = 8, 2048, 768, 8, 96
KT = C // 128        # 6 contraction tiles of 128
TT = T // 128        # 16 t-tiles of 128
NCORES = 8


def _chunks(lo, hi, align=512):
    """Split [lo, hi) at multiples of `align`."""
    out = []
    a = lo
    while a < hi:
        b = min(hi, (a // align + 1) * align)
        out.append((a, b))
        a = b
    return out


def build_nc():
    nc = bacc.Bacc()
    x_b = nc.dram_tensor("x_b", [T, C], F32, kind="ExternalInput")
    wat = nc.dram_tensor("wat", [3 * C, C], F32, kind="ExternalInput")
    wp = nc.dram_tensor("wp", [C, C], F32, kind="ExternalInput")
    ident = nc.dram_tensor("ident", [128, 128], BF16, kind="ExternalInput")
    mk = nc.dram_tensor("mk", [128, 128], F32, kind="ExternalInput")
    bqk = nc.dram_tensor("bqk", [HS, 16], F32, kind="ExternalInput")
    bv = nc.dram_tensor("bv", [128, C], F32, kind="ExternalInput")
    bo = nc.dram_tensor("bo", [128, C], F32, kind="ExternalInput")
    out = nc.dram_tensor("out", [T, C], F32, kind="ExternalOutput")

    with tile.TileContext(nc) as tc, ExitStack() as ctx:
        # Single flat pool scope: tile-pool context exits insert all-engine
        # barriers, which serialized the load/transpose phase against the
        # attention phase. Everything lives in one scope so the scheduler can
        # overlap phases freely; PSUM pools are consolidated to fit 8 banks.
        dram = ctx.enter_context(tc.tile_pool(name="dram", bufs=1, space="DRAM"))
        oT = dram.tile([H, HS, T], BF16, tag="oT")

        consts = ctx.enter_context(tc.tile_pool(name="consts", bufs=1))
        id_sb = consts.tile([128, 128], BF16, tag="id")
        mk_sb = consts.tile([128, 128], F32, tag="mk")
        bqk_sb = consts.tile([HS, 16], F32, tag="bqk")
        bv_sb = consts.tile([128, C], F32, tag="bv")
        bo_sb = consts.tile([128, C], F32, tag="bo")
        # ident first: the very first transposes need it (bf16 so transposes
        # run at 1.0 cycles/row; supplied pre-cast by the host)
        nc.sync.dma_start(id_sb[:], ident[:, :])

        xTp = ctx.enter_context(tc.tile_pool(name="xT", bufs=1))
        xT = xTp.tile([128, KT, T], BF16, tag="xT")
        waTp = ctx.enter_context(tc.tile_pool(name="waT", bufs=1))
        waT = waTp.tile([128, KT, 3 * C], BF16, tag="waT")   # w_attn^T resident
        wpTp = ctx.enter_context(tc.tile_pool(name="wpT", bufs=1))
        wpT_sb = wpTp.tile([128, KT, C], BF16, tag="wpTsb")  # w_proj^T resident

        pin = ctx.enter_context(tc.tile_pool(name="pin", bufs=4))
        vsbp = ctx.enter_context(tc.tile_pool(name="vsb", bufs=2))
        qkp = ctx.enter_context(tc.tile_pool(name="qk", bufs=4))
        ptp = ctx.enter_context(tc.tile_pool(name="pt", bufs=3))
        epp = ctx.enter_context(tc.tile_pool(name="ep", bufs=2))
        pcp = ctx.enter_context(tc.tile_pool(name="pc", bufs=2))
        pco = ctx.enter_context(tc.tile_pool(name="pco", bufs=4))
        # PSUM: pjps 2x[128,512] (transposes + projections), bps 2x[128,1024]
        # (S tiles, epilogue broadcast, phase-C accumulation), opsp 1x[128,1024]
        # (O^T accumulator) = 8 banks exactly.
        pjps = ctx.enter_context(tc.tile_pool(name="pj", bufs=2, space="PSUM"))
        bps = ctx.enter_context(tc.tile_pool(name="bps", bufs=2, space="PSUM"))
        opsp = ctx.enter_context(tc.tile_pool(name="ops", bufs=1, space="PSUM"))

        # ---------------- Loads + transposes ----------------
        # Engines execute their instruction streams IN ORDER, so emission
        # order below is chosen to keep PE saturated: x transposes and the
        # V projection interleave with the streaming loads, head-h+1's QK
        # projection is emitted inside head h's attention, and the w_proj
        # transposes ride along in later heads.
        wat_r = wat.rearrange("(a p) c -> p a c", p=128)
        x_r = x_b.rearrange("(a p) c -> p a c", p=128)
        wp_r = wp.rearrange("(a p) c -> p a c", p=128)

        def tr_group(src, n, kc, dst):
            """Transpose n consecutive 128-blocks of `src` for contraction
            stripe kc into one PSUM tile, then evict (casting to bf16)."""
            psb = bps.tile([128, 1024], BF16, tag="ps", name="tps")
            for i in range(n):
                nc.tensor.matmul(psb[:, i * 128:(i + 1) * 128],
                                 src[:, i, kc * 128:(kc + 1) * 128],
                                 id_sb[:], is_transpose=True,
                                 start=(i == 0), stop=(i == n - 1))
            nc.any.tensor_copy(dst, psb[:, 0:n * 128])

        def tr6(src, n, dst, off):
            for kc in range(KT):
                tr_group(src, n, kc, dst[:, kc, off:off + n * 128])

        def load3(name, src_r, a0, n=3):
            """Load n 128-row blocks [a0, a0+n) of a DRAM tensor."""
            t = pin.tile([128, 3, C], BF16, tag="pin", name=name)
            nc.gpsimd.dma_start(t[:, 0:n, :], src_r[:, a0:a0 + n, :])
            return t

        # DMA queue order = consumption order: w_v first (the V projection
        # starts while x still streams), small consts, then x, then q/k
        # weights, w_proj last.
        x0a = load3("x0a", x_r, 0, 1)
        vch0 = load3("vch0", wat_r, 12)       # w_v for heads 0-3
        nc.sync.dma_start(bv_sb[:], bv[:, :])
        x0b = load3("x0b", x_r, 1, 2)
        xg = [None]
        for g in range(1, 6):
            xg.append(load3(f"x{g}", x_r, 3 * g, 3 if g < 5 else 1))
        nc.sync.dma_start(bqk_sb[:], bqk[:, :])
        nc.sync.dma_start(mk_sb[:], mk[:, :])
        qw0 = load3("qw0", wat_r, 0)
        kw0 = load3("kw0", wat_r, 6)
        vch1 = load3("vch1", wat_r, 15)       # w_v for heads 4-7
        qw1 = load3("qw1", wat_r, 3)
        kw1 = load3("kw1", wat_r, 9)
        nc.sync.dma_start(bo_sb[:], bo[:, :])
        wp0 = load3("wp0", wp_r, 0)
        wp1 = load3("wp1", wp_r, 3)

        # ---------------- Projection / attention helpers ----------------
        Vt = [None, None]

        def v_tile(sb):
            V = vsbp.tile([128, TT, 4, HS + 1], BF16, tag="V")
            nc.vector.memset(V[:, :, :, HS:HS + 1], 1.0)
            Vt[sb] = V

        def v_proj(sb, tts):
            # V = x @ w_v^T + b_v in natural [t, d] layout, 384-wide
            # head-aligned chunks, with an appended ones column per head
            # (softmax denominator).
            V = Vt[sb]
            for tt in tts:
                vps = pjps.tile([128, 512], F32, tag="pj")
                for kc in range(KT):
                    nc.tensor.matmul(vps[:, 0:384],
                                     xT[:, kc, tt * 128:(tt + 1) * 128],
                                     waT[:, kc, 2 * C + 384 * sb:
                                         2 * C + 384 * (sb + 1)],
                                     start=(kc == 0), stop=(kc == KT - 1))
                nc.vector.tensor_tensor(
                    V[:, tt, :, 0:HS],
                    vps[:, 0:384].rearrange("p (h d) -> p h d", d=HS),
                    bv_sb[:, 384 * sb:384 * (sb + 1)]
                        .rearrange("p (h d) -> p h d", d=HS),
                    ADD)

        qk_of = {}

        def qk_proj(h, half=None):
            # Q^T/K^T projection for head h ([d, t] layout, Q pre-scaled by
            # 1/sqrt(hs) via prescaled weights, bias via broadcast add).
            # Emitted in t-halves: attention's first i-half only needs
            # columns 0:1024, so the second half fills its exp-wait windows.
            if half in (None, 0):
                qk_of[h] = [qkp.tile([128, T], BF16, tag="qk", name=f"qk{i}")
                            for i in range(2)]
            qkh = qk_of[h]
            tc4s = range(4) if half is None else (
                range(0, 2) if half == 0 else range(2, 4))
            for tc4 in tc4s:
                for mc in range(2):          # 0 = q, 1 = k
                    wc = h * HS + (0 if mc == 0 else C)
                    pj = pjps.tile([128, 512], F32, tag="pj")
                    for kc in range(KT):
                        nc.tensor.matmul(
                            pj[0:HS, 0:512],
                            waT[:, kc, wc:wc + HS],
                            xT[:, kc, tc4 * 512:(tc4 + 1) * 512],
                            start=(kc == 0), stop=(kc == KT - 1))
                    m_col = h + (0 if mc == 0 else 8)
                    nc.vector.tensor_tensor(
                        qkh[mc][0:HS, tc4 * 512:(tc4 + 1) * 512],
                        pj[0:HS, 0:512],
                        bqk_sb[:, m_col:m_col + 1].to_broadcast([HS, 512]),
                        ADD)

        Oe_of = {}

        def attn_half(h, ihalf, fillers=None):
            # Causal attention in S^T layout: S^T[j, i] via d-contraction;
            # P = exp(S^T) on ACT; diagonal-block mask multiply; O^T (+
            # denominator row l) accumulated in PSUM over j-tiles via
            # lhsT=[V|1]; normalize by 1/l (selector-matmul broadcast).
            qT, kT = qk_of[h]
            V = Vt[h // 4]
            hh = h % 4
            if ihalf == 0:
                Oe_of[h] = epp.tile([HS, T], BF16, tag="Oe", bufs=1,
                                    name=f"Oe{h}")
            Oe = Oe_of[h]
            ibase = 1024 * ihalf
            iend = ibase + 1024
            njt = 8 * (ihalf + 1)
            O_ps = opsp.tile([128, 1024], F32, tag="O")
            # short staircase j-tiles share one S tile / one exp call (ACT
            # dispatch overhead is ~0.3us per activation): the tail of each
            # 8-tile staircase packs as (640+384) and (512+256+128), both
            # exactly fitting a 1024-column tile with bank-legal offsets
            groups = []
            jt = 0
            while jt < njt:
                ilen = iend - max(128 * jt, ibase)
                if ilen == 640 and 128 * jt >= ibase and jt + 4 < njt:
                    groups.append((jt, jt + 2))          # 640 + 384
                    groups.append((jt + 1, jt + 3, jt + 4))  # 512 + 256 + 128
                    jt += 5
                else:
                    groups.append((jt,))
                    jt += 1
            def emit_s(grp):
                # S matmuls + exp + diagonal masks for one j-tile group;
                # returns what the deferred PV stage needs
                S = bps.tile([128, 1024], F32, tag="ps")
                P = ptp.tile([128, 1024], BF16, tag="P")
                parts = []
                off = 0
                for jt in grp:
                    j0 = 128 * jt
                    i0 = max(j0, ibase)
                    ilen = iend - i0
                    for (ra, rb) in _chunks(off, off + ilen):
                        nc.tensor.matmul(
                            S[:, ra:rb],
                            kT[0:HS, j0:j0 + 128],
                            qT[0:HS, i0 + ra - off:i0 + rb - off],
                            start=True, stop=True)
                    parts.append((jt, j0, i0, ilen, off))
                    off += ilen
                nc.scalar.activation(P[:, 0:off], S[:, 0:off], EXP)
                for (jt, j0, i0, ilen, off) in parts:
                    if j0 >= ibase:
                        nc.gpsimd.tensor_tensor(P[:, off:off + 128],
                                                P[:, off:off + 128],
                                                mk_sb[:], MULT)
                return P, parts

            def emit_pv(P, parts):
                for (jt, j0, i0, ilen, off) in parts:
                    for (a, b) in _chunks(i0, iend):
                        ci = a // 512
                        last_jt = min(4 * ci + 3, njt - 1)
                        nc.tensor.matmul(
                            O_ps[0:HS + 1, a - ibase:b - ibase],
                            V[:, jt, hh, :],
                            P[:, off + a - i0:off + b - i0],
                            start=(jt == 0), stop=(jt == last_jt))
                    if fillers and jt in fillers:
                        fillers[jt]()

            def epilogue(ra, rb):
                # normalize [ra, rb) of O_ps by the denominator row
                # (reciprocal of the ones-row sum, broadcast across
                # partitions on gpsimd), then spill that slice
                n = rb - ra
                lt = epp.tile([HS + 1, 1024], F32R, tag="lt", bufs=2)
                nc.vector.tensor_copy(lt[:, 0:n], O_ps[0:HS + 1, ra:rb])
                rr = epp.tile([1, 1024], F32, tag="rr", bufs=2)
                nc.vector.reciprocal(rr[:, 0:n], lt[HS:HS + 1, 0:n])
                R = epp.tile([HS, 1024], F32, tag="R", bufs=2)
                nc.gpsimd.partition_broadcast(R[:, 0:n], rr[:, 0:n])
                nc.gpsimd.tensor_tensor(Oe[:, ibase + ra:ibase + rb],
                                        lt[0:HS, 0:n], R[:, 0:n], MULT)
                # spill eagerly: phase C's stripes unblock while the rest
                # of this head is still computing
                nc.sync.dma_start(oT[h, :, ibase + ra:ibase + rb],
                                  Oe[:, ibase + ra:ibase + rb])

            for grp in groups:
                emit_pv(*emit_s(grp))
            epilogue(0, 1024)

        # ---------------- Output projection (per 2-t-tile block) ----------------
        # O^T streamed back from DRAM as six 128-row contraction stripes
        # (full-partition matmuls: 6 accumulation steps per output chunk
        # instead of 8 96-row ones). Runs on the pj PSUM pool, which is idle
        # once the last projections are done — so tg 0..3 (output rows
        # 0..1024, which only need head 7's first i-half) interleave INTO
        # head 7's ACT-bound second half.
        oT_s = oT.rearrange("h p n -> (h p) n").rearrange("(s q) n -> q s n",
                                                          q=128)
        out_r = out.rearrange("(g a p) c -> p g a c", a=2, p=128)

        def pc_tg(tg):
            otg = pco.tile([128, KT, 256], BF16, tag="otg")
            nc.sync.dma_start(otg[:, 0:5, :],
                              oT_s[:, 0:5, tg * 256:(tg + 1) * 256])
            nc.sync.dma_start(otg[:, 5:6, :],
                              oT_s[:, 5:6, tg * 256:(tg + 1) * 256])
            o_sb = pcp.tile([128, 2, C], F32, tag="osb")
            for ta in range(2):
                for (a, b) in ((0, 512), (512, C)):
                    cps = pjps.tile([128, 512], F32, tag="pj")
                    for s in range(KT):
                        nc.tensor.matmul(cps[:, 0:b - a],
                                         otg[:, s, ta * 128:(ta + 1) * 128],
                                         wpT_sb[:, s, a:b],
                                         start=(s == 0), stop=(s == KT - 1))
                    nc.vector.tensor_tensor(o_sb[:, ta, a:b],
                                            cps[:, 0:b - a],
                                            bo_sb[:, a:b], ADD)
                    if tg == TT // 2 - 1 and ta == 1:
                        # stream the very last tile per chunk: shortens the
                        # final bias-add -> DMA wind-down chain
                        nc.sync.dma_start(out_r[:, tg, ta, a:b],
                                          o_sb[:, ta, a:b])
                if not (tg == TT // 2 - 1 and ta == 1):
                    nc.sync.dma_start(out_r[:, tg, ta], o_sb[:, ta])

        # ---------------- Emission schedule ----------------
        # Lead-in: x transposes interleave with w_v transposes and the V
        # projection of chunks already transposed, so PE tracks the DMA
        # stream with minimal idling.
        v_tile(0)
        # first x block: batch 3 contraction stripes per PSUM tile (single
        # 128-col transposes otherwise ping-pong the evict round-trip)
        for k0 in (0, 3):
            psb = bps.tile([128, 1024], BF16, tag="ps", name="tp0")
            for i in range(3):
                nc.tensor.matmul(psb[:, i * 128:(i + 1) * 128],
                                 x0a[:, 0, (k0 + i) * 128:(k0 + i + 1) * 128],
                                 id_sb[:], is_transpose=True,
                                 start=(i == 0), stop=(i == 2))
            nc.any.tensor_copy(xT[:, k0:k0 + 3, 0:128],
                               psb[:, 0:384].rearrange("p (k c) -> p k c",
                                                       c=128))
        tr6(vch0, 3, waT, 12 * 128)
        v_proj(0, range(0, 1))
        for k0 in (0, 3):
            psb = bps.tile([128, 1024], BF16, tag="ps", name="tp0b")
            for i in range(3):
                for a in range(2):
                    nc.tensor.matmul(
                        psb[:, (2 * i + a) * 128:(2 * i + a + 1) * 128],
                        x0b[:, a, (k0 + i) * 128:(k0 + i + 1) * 128],
                        id_sb[:], is_transpose=True,
                        start=(i == 0 and a == 0), stop=(i == 2 and a == 1))
            nc.any.tensor_copy(xT[:, k0:k0 + 3, 128:384],
                               psb[:, 0:768].rearrange("p (k c) -> p k c",
                                                       c=256))
        v_proj(0, range(1, 3))
        tr6(xg[1], 3, xT, 384)
        v_proj(0, range(3, 6))
        tr6(xg[2], 3, xT, 768)
        v_proj(0, range(6, 9))
        tr6(xg[3], 3, xT, 1152)
        v_proj(0, range(9, 12))
        tr6(xg[4], 3, xT, 1536)
        v_proj(0, range(12, 15))
        tr6(xg[5], 1, xT, 1920)
        v_proj(0, range(15, TT))
        tr6(qw0, 3, waT, 0)
        tr6(kw0, 3, waT, 6 * 128)
        qk_proj(0, 0)

        for h in range(H):
            attn_half(h, 0)
            qk_proj(h, 1)
            if h == 1:
                tr6(vch1, 3, waT, 15 * 128)
                tr6(qw1, 3, waT, 3 * 128)
                tr6(kw1, 3, waT, 9 * 128)
            if h == 2:
                v_tile(1)
                v_proj(1, range(0, 5))
            if h == 3:
                v_proj(1, range(5, 12))
            if h == 4:
                tr6(wp0, 3, wpT_sb, 0)
            if h == 5:
                tr6(wp1, 3, wpT_sb, 384)
            if h == H - 1:
                attn_half(h, 1, fillers={15: lambda: pc_tg(0)})
            else:
                attn_half(h, 1)
            if h == 3:
                v_proj(1, range(12, TT))
            if h < H - 1:
                qk_proj(h + 1, 0)

        # ---------------- Output projection ----------------
        # O^T streamed back from DRAM per 2-t-tile block (pipelined) as six
        # 128-row contraction stripes (full-partition matmuls: 6 accumulation
        # steps per output chunk instead of 8 96-row ones). Stripes 0..4 only
        # depend on heads 0..6, so they prefetch during head 7's attention.
        for tg in range(1, TT // 2):
            pc_tg(tg)

    nc.finalize()
    return nc


_NC_CACHE = {}


def _get_nc():
    if "nc" not in _NC_CACHE:
        _NC_CACHE["nc"] = build_nc()
    return _NC_CACHE["nc"]


def _make_consts(b_attn, b_proj):
    s = 1.0 / math.sqrt(HS)
    bqk = np.empty((HS, 16), dtype=np.float32)
    for m in range(8):
        bqk[:, m] = b_attn[m * HS:(m + 1) * HS] * s
    for m in range(8):
        bqk[:, 8 + m] = b_attn[C + m * HS:C + (m + 1) * HS]
    bv = np.ascontiguousarray(
        np.broadcast_to(b_attn[2 * C:3 * C], (128, C)).astype(np.float32))
    bo = np.ascontiguousarray(
        np.broadcast_to(b_proj, (128, C)).astype(np.float32))
    import ml_dtypes
    ident = np.eye(128, dtype=ml_dtypes.bfloat16)
    mk = np.triu(np.ones((128, 128), dtype=np.float32))
    return bqk, bv, bo, ident, mk


def kernel(x, w_attn, b_attn, w_proj, b_proj, _want_results=False, **run_kwargs):
    x = np.asarray(x, dtype=np.float32)
    w_attn = np.asarray(w_attn, dtype=np.float32)
    b_attn = np.asarray(b_attn, dtype=np.float32)
    w_proj = np.asarray(w_proj, dtype=np.float32)
    b_proj = np.asarray(b_proj, dtype=np.float32)

    s = 1.0 / math.sqrt(HS)
    wat = w_attn.copy()
    wat[0:C, :] *= s            # fold the 1/sqrt(hs) logit scale into Q
    bqk, bv, bo, ident, mk = _make_consts(b_attn, b_proj)

    nc = _get_nc()
    common = dict(wat=wat, wp=w_proj, ident=ident, mk=mk,
                  bqk=bqk, bv=bv, bo=bo)
    in_maps = [dict(x_b=np.ascontiguousarray(x[c]), **common)
               for c in range(NCORES)]
    res = run_bass_kernel_spmd(nc, in_maps, core_ids=list(range(NCORES)),
                               **run_kwargs)
    out = np.stack([res.results[c]["out"] for c in range(NCORES)], axis=0)
    if _want_results:
        return out, res
    return out


if __name__ == "__main__":
    rng = np.random.default_rng(0)
    x = rng.standard_normal((B, T, C), dtype=np.float32)
    w_attn = rng.standard_normal((3 * C, C), dtype=np.float32) / math.sqrt(C)
    b_attn = rng.standard_normal(3 * C).astype(np.float32) * 0.02
    w_proj = rng.standard_normal((C, C), dtype=np.float32) / math.sqrt(C)
    b_proj = rng.standard_normal(C).astype(np.float32) * 0.02
    o = kernel(x, w_attn, b_attn, w_proj, b_proj)
    print("out", o.shape, o.dtype, float(np.abs(o).mean()))

